# revision 18
# baseline (speedup 1.0000x reference)
"""MixLora sparse-MoE Trainium2 kernel (8-core SPMD, expert-parallel).

Sharding: expert-parallel. Core c handles expert e = c // 2, half h = c % 2 of
that expert's routed tokens (top-2 of 4 experts => ~T/4 tokens per expert,
~T/8 per core).  The host computes routing (softmax/top-2/renormalize, a
[T,4] problem - 0.03% of total FLOPs), gathers each core's tokens, and
scatter-adds the per-expert weighted outputs.  Everything else - router
logits, gate/up/down projections, LoRA adapters, SwiGLU, per-token weighting -
runs on device in float32r (full-rate PE, ~1.5e-4 matmul rel-err).

Device layout: activations are kept transposed ([feature partitions, token
free dim]) so no on-chip transposes are needed anywhere:
  LTg  = A_gate^T.T @ Xg^T          [R, C]    (LoRA down-proj, K=D)
  G^T  = Wg^T @ Xg^T + Bg^T @ LTg   [FF, C]   (K=D then K=R accumulate)
  H^T  = silu(G^T + bg) * (U^T+bu)  [FF, C]
  Y^T  = Wd^T @ H^T + Bd^T @ LDT    [D, C]    (K=FF then K=R accumulate)
  out  = (Y^T + bd) * w_tok         [D, C]
LORA_SCALE=2.0 is folded into the B matrices host-side (exact: power of 2).
"""

import sys
import types

import numpy as np

_D = 1024
_FF = 2048
_KC = _D // 128   # k-chunks over D
_FT = _FF // 128  # ff tiles
_R = 16
_E = 4
_NCORES = 8
_LORA_SCALE = 2.0
_CHUNK = 512      # token chunk (one PSUM bank of fp32)


def _install_compat():
    """Patch concourse for this container: walrus here allows only ONE
    sync-wait command per instruction, so split Tile's multi-wait
    instructions into chains of single-wait NOPs on the same engine."""
    import concourse.mybir as mybir
    import concourse.tile as tile_mod
    from concourse.vector_clock import ScopedClock

    if getattr(tile_mod.TileContext._drain_and_barrier, "_wait_split", False):
        return

    _orig_lower = tile_mod.TileContext._lower_ordered_insts

    def _lower_with_wait_split(self, ordered):
        nc = self.nc
        for bb_name in list(ordered.keys()):
            out = []
            for inst in ordered[bb_name]:
                si = inst.sync_info
                waits = list(si.on_wait) if (si is not None and si.on_wait) else []
                if len(waits) > 1:
                    for w in waits[:-1]:
                        out.append(
                            mybir.InstNoOp(
                                name=nc.get_next_instruction_name(),
                                engine=inst.engine,
                                bass_nofuse=True,
                                sync_info=mybir.SyncInfo(on_wait=[w], on_update=[]),
                            )
                        )
                    inst.sync_info = mybir.SyncInfo(
                        on_wait=[waits[-1]], on_update=list(si.on_update or [])
                    )
                out.append(inst)
            ordered[bb_name] = out
        return _orig_lower(self, ordered)

    tile_mod.TileContext._lower_ordered_insts = _lower_with_wait_split

    def _drain_and_barrier(self, tick_clock, wait_clock):
        nc = self.nc
        carrier = nc.sync.nop(nofuse=True, hint="wait_split")
        wait_clock.add_sem_waits(
            carrier.ins, ScopedClock({None: tick_clock.global_clock})
        )
        si = carrier.ins.sync_info
        waits = list(si.on_wait) if si is not None else []
        if len(waits) > 1:
            upd = list(si.on_update) if si.on_update else []
            carrier.ins.sync_info = mybir.SyncInfo(on_wait=[waits[0]], on_update=upd)
            for w in waits[1:]:
                n2 = nc.sync.nop(nofuse=True, hint="wait_split")
                n2.ins.sync_info = mybir.SyncInfo(on_wait=[w], on_update=[])
        drain_inst = nc.sync.drain()
        dsi = drain_inst.ins.sync_info
        if dsi is not None and dsi.on_wait:
            drain_inst.ins.sync_info = mybir.SyncInfo(
                on_wait=[], on_update=list(dsi.on_update or [])
            )
        nc.all_engine_barrier()
        assert self.sems is not None
        popped = nc._tile_sem_poison_stack.pop()
        assert popped is self._sem_poison
        nc.clear_and_free_semaphores(list(self.sems.allocated().values()))
        nc.all_engine_barrier()

    _drain_and_barrier._wait_split = True
    tile_mod.TileContext._drain_and_barrier = _drain_and_barrier

    # Enable walrus redundant-LDWEIGHTS elision: consecutive matmuls that
    # reuse the same stationary operand skip the reload (~175 ns each).
    import concourse.bass_utils as bass_utils

    if not getattr(bass_utils.run_command, "_ldwopt", False):
        _orig_run_command = bass_utils.run_command

        def _run_command_ldwopt(cmd, **kw):
            import os as _os

            if _os.environ.get("BASSMOE_LDWOPT", "0") == "1":
                cmd = [
                    "--enable-ldw-opt=true" if c == "--enable-ldw-opt=false" else c
                    for c in cmd
                ]
            return _orig_run_command(cmd, **kw)

        _run_command_ldwopt._ldwopt = True
        bass_utils.run_command = _run_command_ldwopt


_PROGRAM_CACHE = {}


def _chunks(total, size):
    out = []
    s = 0
    while s < total:
        out.append((s, min(size, total - s)))
        s += size
    return out


def _build_program(C, TL):
    """One SPMD program for all 8 cores. TL = tokens per core for the
    router-logits slice (T / 8)."""
    import concourse.bass as bass
    import concourse.mybir as mybir
    from concourse.tile import TileContext

    dt = mybir.dt
    AF = mybir.ActivationFunctionType
    f32, f32r = dt.float32, dt.float32r

    nc = bass.Bass()
    P = nc.declare_dram_parameter
    xgt = P("xgt", [_KC, 128, C], f32, isOutput=False)
    wg = P("wg", [_FT, 128, _KC, 128], f32, isOutput=False)
    wu = P("wu", [_FT, 128, _KC, 128], f32, isOutput=False)
    wd = P("wd", [_KC, 128, _FT, 128], f32, isOutput=False)
    agut = P("agut", [128, _KC, 48], f32, isOutput=False)
    adt = P("adt", [128, _FT, _R], f32, isOutput=False)
    bgut = P("bgut", [64, _FF], f32, isOutput=False)
    bdt = P("bdt", [_R, _D], f32, isOutput=False)
    gb = P("gb", [128, _FT], f32, isOutput=False)
    ub = P("ub", [128, _FT], f32, isOutput=False)
    db = P("db", [128, _KC], f32, isOutput=False)
    wtok = P("wtok", [128, C], f32, isOutput=False)
    xlt = P("xlt", [128, _KC, TL], f32, isOutput=False)
    gwt = P("gwt", [128, _KC, _E], f32, isOutput=False)
    yt = P("yt", [_KC, 128, C], f32, isOutput=True)
    lgt = P("lgt", [_E, TL], f32, isOutput=True)

    # Equal chunks, all >= 256 so fp32r streams at full rate (1 cyc/row).
    nch = -(-C // _CHUNK)
    csize = -(-C // nch // 32) * 32
    tok_chunks = _chunks(C, csize)

    with TileContext(nc) as tc:
        with (
            tc.tile_pool(name="const", bufs=1) as cp,
            tc.tile_pool(name="wpool", bufs=7) as wp,
            tc.tile_pool(name="wdpool", bufs=2) as wdp,
            tc.tile_pool(name="act", bufs=3) as ap_,
            tc.tile_pool(name="outp", bufs=3) as op_,
            tc.tile_pool(name="pbig", bufs=1, space="PSUM") as pbig,
            tc.tile_pool(name="psmall", bufs=1, space="PSUM") as psmall,
        ):
            # ---- resident loads -------------------------------------------
            t_xgt = []
            for kc in range(_KC):
                tk = cp.tile([128, C], f32r, tag=f"xgt{kc}", name=f"xgt{kc}")
                nc.scalar.dma_start(out=tk[:], in_=xgt[kc].bitcast(f32r))
                t_xgt.append(tk)
            t_gwt = cp.tile([128, _KC, _E], f32r, tag="gwt")
            nc.sync.dma_start(out=t_gwt[:], in_=gwt[:].bitcast(f32r))
            t_agut = cp.tile([128, _KC, 48], f32r, tag="agut")
            nc.sync.dma_start(out=t_agut[:], in_=agut[:].bitcast(f32r))
            t_ltgu = cp.tile([64, C], f32r, tag="ltgu")
            t_ltg = t_ltgu[0:_R, :]
            t_ltu = t_ltgu[32:32 + _R, :]
            t_ldt = cp.tile([_R, C], f32r, tag="ldt")
            t_ht = cp.tile([128, _FT, C], f32r, tag="ht")

            # Two rotating PSUM bank sets (3 banks each): gate/P2/even-d in
            # set A, up/P3.5/odd-d in set B - consumers of one set drain
            # while the PE fills the other, no bank-reuse stalls.
            def psumA(j, w=128):
                return pbig.tile([w, csize], f32, tag=f"ppA{j}", name=f"ppA{j}")

            def psumB(j, w=128):
                return psmall.tile([w, csize], f32, tag=f"ppB{j}", name=f"ppB{j}")

            # ---- P1: router logits slice (x pieces stream via wdpool) -----
            assert TL <= _CHUNK
            pl = psmall.tile([_E, _CHUNK], f32, tag="ppB0", name="ppB0")
            for piece in range((TL + 127) // 128):
                pw = min(128, TL - piece * 128)
                t_xp = wdp.tile([128, _KC, 128], f32r, tag="wd", name="xp")
                nc.gpsimd.dma_start(
                    out=t_xp[:, :, :pw],
                    in_=xlt[:, :, piece * 128:piece * 128 + pw].bitcast(f32r),
                )
                for kc in range(_KC):
                    nc.tensor.matmul(
                        pl[:, piece * 128:piece * 128 + pw],
                        t_gwt[:, kc, :], t_xp[:, kc, :pw],
                        start=(kc == 0), stop=(kc == _KC - 1),
                    )
            s_lg = ap_.tile([_E, TL], f32, tag="sg")
            nc.vector.tensor_copy(s_lg[:], pl[:, :TL])
            nc.sync.dma_start(out=lgt[:], in_=s_lg[:])


            t_wtok = cp.tile([128, C], f32, tag="wtok")
            nc.gpsimd.dma_start(out=t_wtok[:], in_=wtok[:])

            # ---- pre-emit all streaming weight DMAs (in consumption order)
            wgts = []
            for t in range(_FT):
                t_wg = wp.tile([128, _KC, 128], f32r, tag="w", name=f"wg{t}")
                nc.sync.dma_start(out=t_wg[:], in_=wg[t].bitcast(f32r))
                t_wu = wp.tile([128, _KC, 128], f32r, tag="w", name=f"wu{t}")
                nc.sync.dma_start(out=t_wu[:], in_=wu[t].bitcast(f32r))
                wgts.append((t_wg, t_wu))
                if t == 0:
                    # small consts ride behind the first weight pair
                    t_bgut = cp.tile([64, _FF], f32r, tag="bgut")
                    nc.sync.dma_start(out=t_bgut[:], in_=bgut[:].bitcast(f32r))
                    t_adt = cp.tile([128, _FT, _R], f32r, tag="adt")
                    nc.sync.dma_start(out=t_adt[:], in_=adt[:].bitcast(f32r))
                    t_bdt = cp.tile([_R, _D], f32r, tag="bdt")
                    nc.sync.dma_start(out=t_bdt[:], in_=bdt[:].bitcast(f32r))
                    t_gb = cp.tile([128, _FT], f32, tag="gb")
                    nc.sync.dma_start(out=t_gb[:], in_=gb[:])
                    t_ub = cp.tile([128, _FT], f32, tag="ub")
                    nc.sync.dma_start(out=t_ub[:], in_=ub[:])
                    t_db = cp.tile([128, _KC], f32, tag="db")
                    nc.sync.dma_start(out=t_db[:], in_=db[:])
            wds = []
            for d in range(_KC):
                t_wd = wdp.tile([128, _FT, 128], f32r, tag="wd", name=f"wdt{d}")
                nc.sync.dma_start(out=t_wd[:], in_=wd[d].bitcast(f32r))
                wds.append(t_wd)

            # ---- P2: LoRA first stage for gate+up in one pass -------------
            pls = [psumA(j, 48) for j in range(nch)]
            for kc in range(_KC):
                for j, (cs, cw) in enumerate(tok_chunks):
                    nc.tensor.matmul(
                        pls[j][:, :cw], t_agut[:, kc, :], t_xgt[kc][:, cs:cs + cw],
                        start=(kc == 0), stop=(kc == _KC - 1),
                    )
            for j, (cs, cw) in enumerate(tok_chunks):
                nc.vector.tensor_copy(t_ltgu[0:48, cs:cs + cw], pls[j][:, :cw])

            # ---- P3: gate/up + SwiGLU -> H^T (weight-stationary) ----------
            for t in range(_FT):
                t_wg, t_wu = wgts[t]
                pgs = [psumA(j) for j in range(nch)]
                for kc in range(_KC):
                    for j, (cs, cw) in enumerate(tok_chunks):
                        nc.tensor.matmul(
                            pgs[j][:, :cw], t_wg[:, kc, :], t_xgt[kc][:, cs:cs + cw],
                            start=(kc == 0), stop=False,
                        )
                for j, (cs, cw) in enumerate(tok_chunks):
                    nc.tensor.matmul(
                        pgs[j][:, :cw], t_bgut[0:_R, t * 128:(t + 1) * 128],
                        t_ltg[:, cs:cs + cw], start=False, stop=True,
                    )
                sgs = []
                for j, (cs, cw) in enumerate(tok_chunks):
                    sg = ap_.tile([128, csize], f32, tag="sg")
                    nc.scalar.activation(
                        sg[:, :cw], pgs[j][:, :cw], AF.Silu, bias=t_gb[:, t:t + 1]
                    )
                    sgs.append(sg)
                pus = [psumB(j) for j in range(nch)]
                for kc in range(_KC):
                    for j, (cs, cw) in enumerate(tok_chunks):
                        nc.tensor.matmul(
                            pus[j][:, :cw], t_wu[:, kc, :], t_xgt[kc][:, cs:cs + cw],
                            start=(kc == 0), stop=False,
                        )
                for j, (cs, cw) in enumerate(tok_chunks):
                    nc.tensor.matmul(
                        pus[j][:, :cw], t_bgut[32:32 + _R, t * 128:(t + 1) * 128],
                        t_ltu[:, cs:cs + cw], start=False, stop=True,
                    )
                for j, (cs, cw) in enumerate(tok_chunks):
                    # H^T = (U + ub) * silu(G + gb), one DVE op
                    nc.vector.scalar_tensor_tensor(
                        t_ht[:, t, cs:cs + cw], pus[j][:, :cw], t_ub[:, t:t + 1],
                        sgs[j][:, :cw],
                        op0=mybir.AluOpType.add, op1=mybir.AluOpType.mult,
                    )

            # ---- P3.5: LoRA-down first stage over FF ----------------------
            pls = [psumB(j, _R) for j in range(nch)]
            for kf in range(_FT):
                for j, (cs, cw) in enumerate(tok_chunks):
                    nc.tensor.matmul(
                        pls[j][:, :cw], t_adt[:, kf, :], t_ht[:, kf, cs:cs + cw],
                        start=(kf == 0), stop=(kf == _FT - 1),
                    )
            for j, (cs, cw) in enumerate(tok_chunks):
                nc.vector.tensor_copy(t_ldt[:, cs:cs + cw], pls[j][:, :cw])

            # ---- P4: down projection + bias + token weights ---------------
            for d in range(_KC):
                t_wd = wds[d]
                pys = [
                    (psumA if d % 2 == 0 else psumB)(j) for j in range(nch)
                ]
                for kf in range(_FT):
                    for j, (cs, cw) in enumerate(tok_chunks):
                        nc.tensor.matmul(
                            pys[j][:, :cw], t_wd[:, kf, :], t_ht[:, kf, cs:cs + cw],
                            start=(kf == 0), stop=False,
                        )
                for j, (cs, cw) in enumerate(tok_chunks):
                    nc.tensor.matmul(
                        pys[j][:, :cw], t_bdt[:, d * 128:(d + 1) * 128],
                        t_ldt[:, cs:cs + cw], start=False, stop=True,
                    )
                    yo = op_.tile([128, csize], f32, tag="yo")
                    nc.vector.scalar_tensor_tensor(
                        yo[:, :cw], pys[j][:, :cw], t_db[:, d:d + 1],
                        t_wtok[:, cs:cs + cw],
                        op0=mybir.AluOpType.add, op1=mybir.AluOpType.mult,
                    )
                    nc.sync.dma_start(out=yt[d, :, cs:cs + cw], in_=yo[:, :cw])

    return nc


def _pack_dT(a, inner):
    """[D_like, inner] -> [128, D_like/128, inner] with d = kc*128+p."""
    kc = a.shape[0] // 128
    return np.ascontiguousarray(
        a.reshape(kc, 128, inner).transpose(1, 0, 2)
    )


def kernel(**inputs):
    _install_compat()
    from concourse.bass_utils import run_bass_kernel_spmd

    x = np.ascontiguousarray(np.asarray(inputs["x"], dtype=np.float32))
    gate_w = np.asarray(inputs["gate_w"], dtype=np.float32)
    gate_up_w = np.asarray(inputs["gate_up_w"], dtype=np.float32)
    gate_up_b = np.asarray(inputs["gate_up_b"], dtype=np.float32)
    down_w = np.asarray(inputs["down_w"], dtype=np.float32)
    down_b = np.asarray(inputs["down_b"], dtype=np.float32)
    A_gate = np.asarray(inputs["A_gate"], dtype=np.float32)
    B_gate = np.asarray(inputs["B_gate"], dtype=np.float32)
    A_up = np.asarray(inputs["A_up"], dtype=np.float32)
    B_up = np.asarray(inputs["B_up"], dtype=np.float32)
    A_down = np.asarray(inputs["A_down"], dtype=np.float32)
    B_down = np.asarray(inputs["B_down"], dtype=np.float32)
    top_k = int(inputs["top_k"])
    assert top_k == 2, "kernel hardcodes top_k=2"

    T, D = x.shape
    assert D == _D and T % _NCORES == 0
    TL = T // _NCORES

    # ---- host routing (softmax / top-2 / renormalize), fp32 like the ref --
    logits_h = x @ gate_w.T
    ex = np.exp(logits_h - logits_h.max(axis=1, keepdims=True))
    rw = ex / ex.sum(axis=1, keepdims=True)
    sel = np.argsort(-rw, axis=1, kind="stable")[:, :2]
    tw = np.take_along_axis(rw, sel, axis=1)
    tw = tw / tw.sum(axis=1, keepdims=True)

    core_toks = []
    core_wts = []
    for e in range(_E):
        hit = sel == e
        mask = hit.any(axis=1)
        toks = np.nonzero(mask)[0]
        wts = np.where(hit[toks, 0], tw[toks, 0], tw[toks, 1]).astype(np.float32)
        half = (len(toks) + 1) // 2
        core_toks += [toks[:half], toks[half:]]
        core_wts += [wts[:half], wts[half:]]

    C = max(32, -(-max(len(t) for t in core_toks) // 32) * 32)

    key = (C, TL)
    if key not in _PROGRAM_CACHE:
        _PROGRAM_CACHE[key] = _build_program(C, TL)
    nc = _PROGRAM_CACHE[key]

    # ---- per-core input maps ---------------------------------------------
    in_maps = []
    for c in range(_NCORES):
        e = c // 2
        toks = core_toks[c]
        wts = core_wts[c]
        n = len(toks)
        xg = np.zeros((C, _D), np.float32)
        xg[:n] = x[toks]
        wvec = np.zeros(C, np.float32)
        wvec[:n] = wts

        agut = np.zeros((128, _KC, 48), np.float32)
        agut[:, :, 0:_R] = _pack_dT(np.ascontiguousarray(A_gate[e].T), _R)
        agut[:, :, 32:32 + _R] = _pack_dT(np.ascontiguousarray(A_up[e].T), _R)
        bgut = np.zeros((64, _FF), np.float32)
        bgut[0:_R] = B_gate[e].T * _LORA_SCALE
        bgut[32:32 + _R] = B_up[e].T * _LORA_SCALE

        wgu = gate_up_w[e]  # [D, 2FF]
        m = {
            "xgt": np.ascontiguousarray(xg.T.reshape(_KC, 128, C)),
            "wg": np.ascontiguousarray(
                wgu[:, :_FF].reshape(_KC, 128, _FT, 128).transpose(2, 1, 0, 3)
            ),
            "wu": np.ascontiguousarray(
                wgu[:, _FF:].reshape(_KC, 128, _FT, 128).transpose(2, 1, 0, 3)
            ),
            "wd": np.ascontiguousarray(
                down_w[e].reshape(_FT, 128, _KC, 128).transpose(2, 1, 0, 3)
            ),
            "agut": agut,
            "adt": _pack_dT(np.ascontiguousarray(A_down[e].T), _R),
            "bgut": bgut,
            "bdt": np.ascontiguousarray(B_down[e].T) * _LORA_SCALE,
            "gb": np.ascontiguousarray(gate_up_b[e, :_FF].reshape(_FT, 128).T),
            "ub": np.ascontiguousarray(gate_up_b[e, _FF:].reshape(_FT, 128).T),
            "db": np.ascontiguousarray(down_b[e].reshape(_KC, 128).T),
            "wtok": np.ascontiguousarray(np.broadcast_to(wvec, (128, C))),
            "xlt": _pack_dT(
                np.ascontiguousarray(x[c * TL:(c + 1) * TL].T), TL
            ),
            "gwt": _pack_dT(np.ascontiguousarray(gate_w.T), _E),
        }
        in_maps.append(m)

    trace = bool(int(__import__("os").environ.get("BASSMOE_TRACE", "0")))
    kwargs = {}
    if trace:
        kwargs = dict(trace=True, trace_cores=list(range(_NCORES)))
    res = run_bass_kernel_spmd(nc, in_maps, core_ids=list(range(_NCORES)), **kwargs)
    sys.modules[__name__]._last_results = res

    # ---- unshard: scatter-add expert contributions, assemble logits -------
    final = np.zeros((T, _D), np.float32)
    for e in range(_E):
        for h in range(2):
            c = 2 * e + h
            toks = core_toks[c]
            if len(toks) == 0:
                continue
            yt = res.results[c]["yt"].reshape(_D, C)
            final[toks] += yt[:, :len(toks)].T
    router_logits = np.concatenate(
        [res.results[c]["lgt"].T for c in range(_NCORES)], axis=0
    )
    return final, router_logits


# revision 19
# speedup vs baseline: 1.0480x; 1.0480x over previous
"""MixLora sparse-MoE Trainium2 kernel (8-core SPMD, expert-parallel).

Sharding: expert-parallel. Core c handles expert e = c // 2, half h = c % 2 of
that expert's routed tokens (top-2 of 4 experts => ~T/4 tokens per expert,
~T/8 per core).  The host computes routing (softmax/top-2/renormalize, a
[T,4] problem - 0.03% of total FLOPs), gathers each core's tokens, and
scatter-adds the per-expert weighted outputs.  Everything else - router
logits, gate/up/down projections, LoRA adapters, SwiGLU, per-token weighting -
runs on device in float32r (full-rate PE, ~1.5e-4 matmul rel-err).

Device layout: activations are kept transposed ([feature partitions, token
free dim]) so no on-chip transposes are needed anywhere:
  LTg  = A_gate^T.T @ Xg^T          [R, C]    (LoRA down-proj, K=D)
  G^T  = Wg^T @ Xg^T + Bg^T @ LTg   [FF, C]   (K=D then K=R accumulate)
  H^T  = silu(G^T + bg) * (U^T+bu)  [FF, C]
  Y^T  = Wd^T @ H^T + Bd^T @ LDT    [D, C]    (K=FF then K=R accumulate)
  out  = (Y^T + bd) * w_tok         [D, C]
LORA_SCALE=2.0 is folded into the B matrices host-side (exact: power of 2).
"""

import sys
import types

import numpy as np

_D = 1024
_FF = 2048
_KC = _D // 128   # k-chunks over D
_FT = _FF // 128  # ff tiles
_R = 16
_E = 4
_NCORES = 8
_LORA_SCALE = 2.0
_CHUNK = 512      # token chunk (one PSUM bank of fp32)


def _install_compat():
    """Patch concourse for this container: walrus here allows only ONE
    sync-wait command per instruction, so split Tile's multi-wait
    instructions into chains of single-wait NOPs on the same engine."""
    import concourse.mybir as mybir
    import concourse.tile as tile_mod
    from concourse.vector_clock import ScopedClock

    if getattr(tile_mod.TileContext._drain_and_barrier, "_wait_split", False):
        return

    _orig_lower = tile_mod.TileContext._lower_ordered_insts

    def _lower_with_wait_split(self, ordered):
        nc = self.nc
        for bb_name in list(ordered.keys()):
            out = []
            for inst in ordered[bb_name]:
                si = inst.sync_info
                waits = list(si.on_wait) if (si is not None and si.on_wait) else []
                if len(waits) > 1:
                    for w in waits[:-1]:
                        out.append(
                            mybir.InstNoOp(
                                name=nc.get_next_instruction_name(),
                                engine=inst.engine,
                                bass_nofuse=True,
                                sync_info=mybir.SyncInfo(on_wait=[w], on_update=[]),
                            )
                        )
                    inst.sync_info = mybir.SyncInfo(
                        on_wait=[waits[-1]], on_update=list(si.on_update or [])
                    )
                out.append(inst)
            ordered[bb_name] = out
        return _orig_lower(self, ordered)

    tile_mod.TileContext._lower_ordered_insts = _lower_with_wait_split

    def _drain_and_barrier(self, tick_clock, wait_clock):
        nc = self.nc
        carrier = nc.sync.nop(nofuse=True, hint="wait_split")
        wait_clock.add_sem_waits(
            carrier.ins, ScopedClock({None: tick_clock.global_clock})
        )
        si = carrier.ins.sync_info
        waits = list(si.on_wait) if si is not None else []
        if len(waits) > 1:
            upd = list(si.on_update) if si.on_update else []
            carrier.ins.sync_info = mybir.SyncInfo(on_wait=[waits[0]], on_update=upd)
            for w in waits[1:]:
                n2 = nc.sync.nop(nofuse=True, hint="wait_split")
                n2.ins.sync_info = mybir.SyncInfo(on_wait=[w], on_update=[])
        drain_inst = nc.sync.drain()
        dsi = drain_inst.ins.sync_info
        if dsi is not None and dsi.on_wait:
            drain_inst.ins.sync_info = mybir.SyncInfo(
                on_wait=[], on_update=list(dsi.on_update or [])
            )
        nc.all_engine_barrier()
        assert self.sems is not None
        popped = nc._tile_sem_poison_stack.pop()
        assert popped is self._sem_poison
        nc.clear_and_free_semaphores(list(self.sems.allocated().values()))
        nc.all_engine_barrier()

    _drain_and_barrier._wait_split = True
    tile_mod.TileContext._drain_and_barrier = _drain_and_barrier

    # Enable walrus redundant-LDWEIGHTS elision: consecutive matmuls that
    # reuse the same stationary operand skip the reload (~175 ns each).
    import concourse.bass_utils as bass_utils

    if not getattr(bass_utils.run_command, "_ldwopt", False):
        _orig_run_command = bass_utils.run_command

        def _run_command_ldwopt(cmd, **kw):
            import os as _os

            if _os.environ.get("BASSMOE_LDWOPT", "0") == "1":
                cmd = [
                    "--enable-ldw-opt=true" if c == "--enable-ldw-opt=false" else c
                    for c in cmd
                ]
            return _orig_run_command(cmd, **kw)

        _run_command_ldwopt._ldwopt = True
        bass_utils.run_command = _run_command_ldwopt


_PROGRAM_CACHE = {}


def _chunks(total, size):
    out = []
    s = 0
    while s < total:
        out.append((s, min(size, total - s)))
        s += size
    return out


def _build_program(C, TL):
    """One SPMD program for all 8 cores. TL = tokens per core for the
    router-logits slice (T / 8)."""
    import concourse.bass as bass
    import concourse.mybir as mybir
    from concourse.tile import TileContext

    dt = mybir.dt
    AF = mybir.ActivationFunctionType
    f32, f32r = dt.float32, dt.float32r

    nc = bass.Bass()
    P = nc.declare_dram_parameter
    xgt = P("xgt", [_KC, 128, C], f32, isOutput=False)
    wg = P("wg", [_FT, 128, _KC, 128], f32, isOutput=False)
    wu = P("wu", [_FT, 128, _KC, 128], f32, isOutput=False)
    wd = P("wd", [_KC, 128, _FT, 128], f32, isOutput=False)
    agut = P("agut", [128, _KC, 48], f32, isOutput=False)
    adt = P("adt", [128, _FT, _R], f32, isOutput=False)
    bgut = P("bgut", [64, _FF], f32, isOutput=False)
    bdt = P("bdt", [_R, _D], f32, isOutput=False)
    gb = P("gb", [128, _FT], f32, isOutput=False)
    ub = P("ub", [128, _FT], f32, isOutput=False)
    db = P("db", [128, _KC], f32, isOutput=False)
    wtok = P("wtok", [128, C], f32, isOutput=False)
    xlt = P("xlt", [128, _KC, TL], f32, isOutput=False)
    gwt = P("gwt", [128, _KC, _E], f32, isOutput=False)
    yt = P("yt", [_KC, 128, C], f32, isOutput=True)
    lgt = P("lgt", [_E, TL], f32, isOutput=True)

    # Equal chunks, all >= 256 so fp32r streams at full rate (1 cyc/row).
    nch = -(-C // _CHUNK)
    csize = -(-C // nch // 32) * 32
    tok_chunks = _chunks(C, csize)

    with TileContext(nc) as tc:
        with (
            tc.tile_pool(name="const", bufs=1) as cp,
            tc.tile_pool(name="wpool", bufs=7) as wp,
            tc.tile_pool(name="wdpool", bufs=2) as wdp,
            tc.tile_pool(name="act", bufs=3) as ap_,
            tc.tile_pool(name="outp", bufs=3) as op_,
            tc.tile_pool(name="pbig", bufs=1, space="PSUM") as pbig,
            tc.tile_pool(name="psmall", bufs=1, space="PSUM") as psmall,
        ):
            # ---- resident loads -------------------------------------------
            t_xgt = []
            for kc in range(_KC):
                tk = cp.tile([128, C], f32r, tag=f"xgt{kc}", name=f"xgt{kc}")
                nc.scalar.dma_start(out=tk[:], in_=xgt[kc].bitcast(f32r))
                t_xgt.append(tk)
            t_gwt = cp.tile([128, _KC, _E], f32r, tag="gwt")
            nc.sync.dma_start(out=t_gwt[:], in_=gwt[:].bitcast(f32r))
            t_agut = cp.tile([128, _KC, 48], f32r, tag="agut")
            nc.sync.dma_start(out=t_agut[:], in_=agut[:].bitcast(f32r))
            t_ltgu = cp.tile([64, C], f32r, tag="ltgu")
            t_ltg = t_ltgu[0:_R, :]
            t_ltu = t_ltgu[32:32 + _R, :]
            t_ldt = cp.tile([_R, C], f32r, tag="ldt")
            t_ht = cp.tile([128, _FT, C], f32r, tag="ht")

            # Two rotating PSUM bank sets (3 banks each): gate/P2/even-d in
            # set A, up/P3.5/odd-d in set B - consumers of one set drain
            # while the PE fills the other, no bank-reuse stalls.
            def psumA(j, w=128):
                return pbig.tile([w, csize], f32, tag=f"ppA{j}", name=f"ppA{j}")

            def psumB(j, w=128):
                return psmall.tile([w, csize], f32, tag=f"ppB{j}", name=f"ppB{j}")

            # ---- P1: router logits slice (x pieces stream via wdpool) -----
            assert TL <= _CHUNK
            pl = psmall.tile([_E, _CHUNK], f32, tag="ppB0", name="ppB0")
            for piece in range((TL + 127) // 128):
                pw = min(128, TL - piece * 128)
                t_xp = wdp.tile([128, _KC, 128], f32r, tag="wd", name="xp")
                nc.sync.dma_start(
                    out=t_xp[:, :, :pw],
                    in_=xlt[:, :, piece * 128:piece * 128 + pw].bitcast(f32r),
                )
                for kc in range(_KC):
                    nc.tensor.matmul(
                        pl[:, piece * 128:piece * 128 + pw],
                        t_gwt[:, kc, :], t_xp[:, kc, :pw],
                        start=(kc == 0), stop=(kc == _KC - 1),
                    )
            s_lg = ap_.tile([_E, TL], f32, tag="sg")
            nc.vector.tensor_copy(s_lg[:], pl[:, :TL])
            nc.sync.dma_start(out=lgt[:], in_=s_lg[:])


            t_wtok = cp.tile([128, C], f32, tag="wtok")
            nc.gpsimd.dma_start(out=t_wtok[:], in_=wtok[:])

            # ---- pre-emit all streaming weight DMAs (in consumption order)
            wgts = []
            for t in range(_FT):
                t_wg = wp.tile([128, _KC, 128], f32r, tag="w", name=f"wg{t}")
                nc.sync.dma_start(out=t_wg[:], in_=wg[t].bitcast(f32r))
                t_wu = wp.tile([128, _KC, 128], f32r, tag="w", name=f"wu{t}")
                nc.sync.dma_start(out=t_wu[:], in_=wu[t].bitcast(f32r))
                wgts.append((t_wg, t_wu))
                if t == 0:
                    # small consts ride behind the first weight pair
                    t_bgut = cp.tile([64, _FF], f32r, tag="bgut")
                    nc.sync.dma_start(out=t_bgut[:], in_=bgut[:].bitcast(f32r))
                    t_adt = cp.tile([128, _FT, _R], f32r, tag="adt")
                    nc.sync.dma_start(out=t_adt[:], in_=adt[:].bitcast(f32r))
                    t_bdt = cp.tile([_R, _D], f32r, tag="bdt")
                    nc.sync.dma_start(out=t_bdt[:], in_=bdt[:].bitcast(f32r))
                    t_gb = cp.tile([128, _FT], f32, tag="gb")
                    nc.sync.dma_start(out=t_gb[:], in_=gb[:])
                    t_ub = cp.tile([128, _FT], f32, tag="ub")
                    nc.sync.dma_start(out=t_ub[:], in_=ub[:])
                    t_db = cp.tile([128, _KC], f32, tag="db")
                    nc.sync.dma_start(out=t_db[:], in_=db[:])
            wds = []
            for d in range(_KC):
                t_wd = wdp.tile([128, _FT, 128], f32r, tag="wd", name=f"wdt{d}")
                nc.sync.dma_start(out=t_wd[:], in_=wd[d].bitcast(f32r))
                wds.append(t_wd)

            # ---- P2: LoRA first stage for gate+up in one pass -------------
            pls = [psumA(j, 48) for j in range(nch)]
            for kc in range(_KC):
                for j, (cs, cw) in enumerate(tok_chunks):
                    nc.tensor.matmul(
                        pls[j][:, :cw], t_agut[:, kc, :], t_xgt[kc][:, cs:cs + cw],
                        start=(kc == 0), stop=(kc == _KC - 1),
                    )
            for j, (cs, cw) in enumerate(tok_chunks):
                nc.vector.tensor_copy(t_ltgu[0:48, cs:cs + cw], pls[j][:, :cw])

            # ---- P3: gate/up + SwiGLU -> H^T (weight-stationary) ----------
            for t in range(_FT):
                t_wg, t_wu = wgts[t]
                pgs = [psumA(j) for j in range(nch)]
                for kc in range(_KC):
                    for j, (cs, cw) in enumerate(tok_chunks):
                        nc.tensor.matmul(
                            pgs[j][:, :cw], t_wg[:, kc, :], t_xgt[kc][:, cs:cs + cw],
                            start=(kc == 0), stop=False,
                        )
                for j, (cs, cw) in enumerate(tok_chunks):
                    nc.tensor.matmul(
                        pgs[j][:, :cw], t_bgut[0:_R, t * 128:(t + 1) * 128],
                        t_ltg[:, cs:cs + cw], start=False, stop=True,
                    )
                sgs = []
                for j, (cs, cw) in enumerate(tok_chunks):
                    sg = ap_.tile([128, csize], f32, tag="sg")
                    nc.scalar.activation(
                        sg[:, :cw], pgs[j][:, :cw], AF.Silu, bias=t_gb[:, t:t + 1]
                    )
                    sgs.append(sg)
                pus = [psumB(j) for j in range(nch)]
                for kc in range(_KC):
                    for j, (cs, cw) in enumerate(tok_chunks):
                        nc.tensor.matmul(
                            pus[j][:, :cw], t_wu[:, kc, :], t_xgt[kc][:, cs:cs + cw],
                            start=(kc == 0), stop=False,
                        )
                for j, (cs, cw) in enumerate(tok_chunks):
                    nc.tensor.matmul(
                        pus[j][:, :cw], t_bgut[32:32 + _R, t * 128:(t + 1) * 128],
                        t_ltu[:, cs:cs + cw], start=False, stop=True,
                    )
                for j, (cs, cw) in enumerate(tok_chunks):
                    # H^T = (U + ub) * silu(G + gb), one DVE op
                    nc.vector.scalar_tensor_tensor(
                        t_ht[:, t, cs:cs + cw], pus[j][:, :cw], t_ub[:, t:t + 1],
                        sgs[j][:, :cw],
                        op0=mybir.AluOpType.add, op1=mybir.AluOpType.mult,
                    )

            # ---- P3.5: LoRA-down first stage over FF ----------------------
            pls = [psumB(j, _R) for j in range(nch)]
            for kf in range(_FT):
                for j, (cs, cw) in enumerate(tok_chunks):
                    nc.tensor.matmul(
                        pls[j][:, :cw], t_adt[:, kf, :], t_ht[:, kf, cs:cs + cw],
                        start=(kf == 0), stop=(kf == _FT - 1),
                    )
            for j, (cs, cw) in enumerate(tok_chunks):
                nc.vector.tensor_copy(t_ldt[:, cs:cs + cw], pls[j][:, :cw])

            # ---- P4: down projection + bias + token weights ---------------
            for d in range(_KC):
                t_wd = wds[d]
                pys = [
                    (psumA if d % 2 == 0 else psumB)(j) for j in range(nch)
                ]
                for kf in range(_FT):
                    for j, (cs, cw) in enumerate(tok_chunks):
                        nc.tensor.matmul(
                            pys[j][:, :cw], t_wd[:, kf, :], t_ht[:, kf, cs:cs + cw],
                            start=(kf == 0), stop=False,
                        )
                for j, (cs, cw) in enumerate(tok_chunks):
                    nc.tensor.matmul(
                        pys[j][:, :cw], t_bdt[:, d * 128:(d + 1) * 128],
                        t_ldt[:, cs:cs + cw], start=False, stop=True,
                    )
                    yo = op_.tile([128, csize], f32, tag="yo")
                    nc.vector.scalar_tensor_tensor(
                        yo[:, :cw], pys[j][:, :cw], t_db[:, d:d + 1],
                        t_wtok[:, cs:cs + cw],
                        op0=mybir.AluOpType.add, op1=mybir.AluOpType.mult,
                    )
                    nc.sync.dma_start(out=yt[d, :, cs:cs + cw], in_=yo[:, :cw])

    return nc


def _pack_dT(a, inner):
    """[D_like, inner] -> [128, D_like/128, inner] with d = kc*128+p."""
    kc = a.shape[0] // 128
    return np.ascontiguousarray(
        a.reshape(kc, 128, inner).transpose(1, 0, 2)
    )


def kernel(**inputs):
    _install_compat()
    from concourse.bass_utils import run_bass_kernel_spmd

    x = np.ascontiguousarray(np.asarray(inputs["x"], dtype=np.float32))
    gate_w = np.asarray(inputs["gate_w"], dtype=np.float32)
    gate_up_w = np.asarray(inputs["gate_up_w"], dtype=np.float32)
    gate_up_b = np.asarray(inputs["gate_up_b"], dtype=np.float32)
    down_w = np.asarray(inputs["down_w"], dtype=np.float32)
    down_b = np.asarray(inputs["down_b"], dtype=np.float32)
    A_gate = np.asarray(inputs["A_gate"], dtype=np.float32)
    B_gate = np.asarray(inputs["B_gate"], dtype=np.float32)
    A_up = np.asarray(inputs["A_up"], dtype=np.float32)
    B_up = np.asarray(inputs["B_up"], dtype=np.float32)
    A_down = np.asarray(inputs["A_down"], dtype=np.float32)
    B_down = np.asarray(inputs["B_down"], dtype=np.float32)
    top_k = int(inputs["top_k"])
    assert top_k == 2, "kernel hardcodes top_k=2"

    T, D = x.shape
    assert D == _D and T % _NCORES == 0
    TL = T // _NCORES

    # ---- host routing (softmax / top-2 / renormalize), fp32 like the ref --
    logits_h = x @ gate_w.T
    ex = np.exp(logits_h - logits_h.max(axis=1, keepdims=True))
    rw = ex / ex.sum(axis=1, keepdims=True)
    sel = np.argsort(-rw, axis=1, kind="stable")[:, :2]
    tw = np.take_along_axis(rw, sel, axis=1)
    tw = tw / tw.sum(axis=1, keepdims=True)

    core_toks = []
    core_wts = []
    for e in range(_E):
        hit = sel == e
        mask = hit.any(axis=1)
        toks = np.nonzero(mask)[0]
        wts = np.where(hit[toks, 0], tw[toks, 0], tw[toks, 1]).astype(np.float32)
        half = (len(toks) + 1) // 2
        core_toks += [toks[:half], toks[half:]]
        core_wts += [wts[:half], wts[half:]]

    C = max(32, -(-max(len(t) for t in core_toks) // 32) * 32)

    key = (C, TL)
    if key not in _PROGRAM_CACHE:
        _PROGRAM_CACHE[key] = _build_program(C, TL)
    nc = _PROGRAM_CACHE[key]

    # ---- per-core input maps ---------------------------------------------
    in_maps = []
    for c in range(_NCORES):
        e = c // 2
        toks = core_toks[c]
        wts = core_wts[c]
        n = len(toks)
        xg = np.zeros((C, _D), np.float32)
        xg[:n] = x[toks]
        wvec = np.zeros(C, np.float32)
        wvec[:n] = wts

        agut = np.zeros((128, _KC, 48), np.float32)
        agut[:, :, 0:_R] = _pack_dT(np.ascontiguousarray(A_gate[e].T), _R)
        agut[:, :, 32:32 + _R] = _pack_dT(np.ascontiguousarray(A_up[e].T), _R)
        bgut = np.zeros((64, _FF), np.float32)
        bgut[0:_R] = B_gate[e].T * _LORA_SCALE
        bgut[32:32 + _R] = B_up[e].T * _LORA_SCALE

        wgu = gate_up_w[e]  # [D, 2FF]
        m = {
            "xgt": np.ascontiguousarray(xg.T.reshape(_KC, 128, C)),
            "wg": np.ascontiguousarray(
                wgu[:, :_FF].reshape(_KC, 128, _FT, 128).transpose(2, 1, 0, 3)
            ),
            "wu": np.ascontiguousarray(
                wgu[:, _FF:].reshape(_KC, 128, _FT, 128).transpose(2, 1, 0, 3)
            ),
            "wd": np.ascontiguousarray(
                down_w[e].reshape(_FT, 128, _KC, 128).transpose(2, 1, 0, 3)
            ),
            "agut": agut,
            "adt": _pack_dT(np.ascontiguousarray(A_down[e].T), _R),
            "bgut": bgut,
            "bdt": np.ascontiguousarray(B_down[e].T) * _LORA_SCALE,
            "gb": np.ascontiguousarray(gate_up_b[e, :_FF].reshape(_FT, 128).T),
            "ub": np.ascontiguousarray(gate_up_b[e, _FF:].reshape(_FT, 128).T),
            "db": np.ascontiguousarray(down_b[e].reshape(_KC, 128).T),
            "wtok": np.ascontiguousarray(np.broadcast_to(wvec, (128, C))),
            "xlt": _pack_dT(
                np.ascontiguousarray(x[c * TL:(c + 1) * TL].T), TL
            ),
            "gwt": _pack_dT(np.ascontiguousarray(gate_w.T), _E),
        }
        in_maps.append(m)

    trace = bool(int(__import__("os").environ.get("BASSMOE_TRACE", "0")))
    kwargs = {}
    if trace:
        kwargs = dict(trace=True, trace_cores=list(range(_NCORES)))
    res = run_bass_kernel_spmd(nc, in_maps, core_ids=list(range(_NCORES)), **kwargs)
    sys.modules[__name__]._last_results = res

    # ---- unshard: scatter-add expert contributions, assemble logits -------
    final = np.zeros((T, _D), np.float32)
    for e in range(_E):
        for h in range(2):
            c = 2 * e + h
            toks = core_toks[c]
            if len(toks) == 0:
                continue
            yt = res.results[c]["yt"].reshape(_D, C)
            final[toks] += yt[:, :len(toks)].T
    router_logits = np.concatenate(
        [res.results[c]["lgt"].T for c in range(_NCORES)], axis=0
    )
    return final, router_logits
